# revision 7
# baseline (speedup 1.0000x reference)
"""Trainium2 Bass kernel for nn_Conv1dAttention.

Math (per sample):
  q,k,v,pe = lrelu(bn(conv1d(x, W_p)))           # [C=128, L=2048], Cin=64, K=3
  S = q^T k                                      # [L, L]
  P = softmax_rows(S)                            # softmax over last axis
  out = v @ P + pe                               # [C, L]

Sharding: data-parallel over batch B=16 across 8 NeuronCores (2 samples/core).
Same NEFF on all cores, per-core input shards, no collectives.

Design notes:
  - BN (uses given mean/var, not data stats) is folded into conv weights and
    bias on the host. Bias is injected via an appended ones-row in the im2col
    tile, so conv+bias is pure matmul.
  - im2col: contraction 192 = two chunks: chunk1 = 128 rows (k=0 shifted,
    k=1 center), chunk2 = 65 rows (k=2 shifted + ones row for bias).
  - Q, K, PE computed in [c, l] layout (N=512 matmuls). V computed directly
    transposed [l, c] (stationary = im2col slice) to feed the V@P matmul
    without PE transposes.
  - Softmax without max subtraction (logits bounded ~60 for this model's
    weight scale; exp stays finite in fp32). exp on ScalarE with accum_out
    giving row sums; 1/Z folded into the tiny [128,128] V^T block on DVE.
  - All matmuls in float32r (FP22, 1 cycle/row) — ample precision here.
  - PSUM: 4 banks = [128,2048] output accumulator; 4 banks = two rotating
    [128,1024] tiles shared by convs and S-chunks.
"""

import sys

if "/opt/trn_rl_repo" not in sys.path:
    sys.path.insert(0, "/opt/trn_rl_repo")

from contextlib import ExitStack

import numpy as np

import concourse.bass as bass
import concourse.tile as tile
from concourse import bacc, mybir
from concourse.bass_utils import run_bass_kernel_spmd

B, CIN, COUT, KW, L = 16, 64, 128, 3, 2048
NCORES = 8
BP = B // NCORES  # samples per core
EPS = 1e-5
SLOPE = 0.3
F32 = mybir.dt.float32
F32R = mybir.dt.float32r
NB = L // 128  # 16 a-blocks
HALF = 1024

_CACHE = {}


def _r(ap):
    return ap.bitcast(F32R)


def _body(ctx, tc, x, w1, w2, zc, onesrow, out):
    nc = tc.nc
    mult = mybir.AluOpType.mult
    amax = mybir.AluOpType.max
    Exp = mybir.ActivationFunctionType.Exp

    wpool = ctx.enter_context(tc.tile_pool(name="wpool", bufs=1))
    xpool = ctx.enter_context(tc.tile_pool(name="xpool", bufs=2))
    apool = ctx.enter_context(tc.tile_pool(name="apool", bufs=2))
    ppool = ctx.enter_context(tc.tile_pool(name="ppool", bufs=2))
    opool = ctx.enter_context(tc.tile_pool(name="opool", bufs=2))
    vpool = ctx.enter_context(tc.tile_pool(name="vpool", bufs=2))
    zpool = ctx.enter_context(tc.tile_pool(name="zpool", bufs=4))
    lpool = ctx.enter_context(tc.tile_pool(name="lpool", bufs=2))
    psA = ctx.enter_context(tc.tile_pool(name="psA", bufs=2, space="PSUM"))
    psO = ctx.enter_context(tc.tile_pool(name="psO", bufs=1, space="PSUM"))

    w1_t, w2_t = {}, {}
    for p in "qkvp":
        w1_t[p] = wpool.tile([128, COUT], F32R, tag=f"w1{p}", name=f"w1{p}")
        nc.sync.dma_start(out=w1_t[p][:, :], in_=w1[p][:, :].bitcast(F32R))
        w2_t[p] = wpool.tile([CIN + 1, COUT], F32R, tag=f"w2{p}", name=f"w2{p}")
        nc.sync.dma_start(out=w2_t[p][:, :], in_=w2[p][:, :].bitcast(F32R))

    for s in range(BP):
        # im2col tiles.
        # xs1 rows 0-63  = x[cin, l-1]  (k=0), rows 64-127 = x[cin, l] (k=1)
        # xs2 rows 0-63  = x[cin, l+1]  (k=2), row 64 = ones (bias)
        xs1 = xpool.tile([128, L], F32R, tag="xs1", name="xs1")
        nc.sync.dma_start(out=xs1[0:CIN, 1:L], in_=x[s, :, 0 : L - 1].bitcast(F32R))
        nc.sync.dma_start(out=xs1[0:CIN, 0:1], in_=zc[:, :].bitcast(F32R))
        nc.sync.dma_start(out=xs1[CIN:128, 0:L], in_=x[s, :, :].bitcast(F32R))
        xs2 = xpool.tile([CIN + 1, L], F32R, tag="xs2", name="xs2")
        nc.sync.dma_start(out=xs2[0:CIN, 0 : L - 1], in_=x[s, :, 1:L].bitcast(F32R))
        nc.sync.dma_start(out=xs2[0:CIN, L - 1 : L], in_=zc[:, :].bitcast(F32R))
        nc.sync.dma_start(out=xs2[CIN : CIN + 1, :], in_=onesrow[:, :].bitcast(F32R))

        # Q, K, PE convs in [c, l] layout: lhsT = weights, rhs = im2col.
        acts = {}
        for p in ("q", "k", "p"):
            dst = apool.tile([128, L], F32R, tag=f"act{p}", name=f"act{p}")
            for h in range(2):
                cps = psA.tile([128, HALF], F32, tag="ps", name="cps")
                for n in range(2):
                    c0 = h * HALF + n * 512
                    nc.tensor.matmul(
                        cps[:, n * 512 : n * 512 + 512],
                        w1_t[p][:, :],
                        xs1[:, c0 : c0 + 512],
                        start=True,
                        stop=False,
                    )
                    nc.tensor.matmul(
                        cps[:, n * 512 : n * 512 + 512],
                        w2_t[p][:, :],
                        xs2[:, c0 : c0 + 512],
                        start=False,
                        stop=True,
                    )
                # lrelu(y) = max(0.3*y, y): 2 DVE passes (one PSUM read each)
                lt = lpool.tile([128, HALF], F32, tag="lt", name="lt")
                nc.vector.tensor_scalar_mul(lt[:, :], cps[:, :], SLOPE)
                nc.vector.tensor_tensor(
                    dst[:, h * HALF : (h + 1) * HALF], cps[:, :], lt[:, :], amax
                )
            acts[p] = dst

        # V directly transposed: [l, c] blocks; lhsT = im2col slice, rhs = W.
        vt = apool.tile([128, L], F32, tag="vt", name="vt")  # col = blk*128 + c
        for g in range(2):
            vps = psA.tile([128, HALF], F32, tag="ps", name="vps")
            for i in range(8):
                blk = g * 8 + i
                lsl = slice(blk * 128, blk * 128 + 128)
                pc = slice(i * 128, i * 128 + 128)
                nc.tensor.matmul(
                    vps[:, pc],
                    xs1[:, lsl],
                    w1_t["v"][:, :],
                    start=True,
                    stop=False,
                )
                nc.tensor.matmul(
                    vps[:, pc],
                    xs2[:, lsl],
                    w2_t["v"][:, :],
                    start=False,
                    stop=True,
                )
            lt = lpool.tile([128, HALF], F32, tag="lt", name="lt")
            nc.vector.tensor_scalar_mul(lt[:, :], vps[:, :], SLOPE)
            nc.vector.tensor_tensor(
                vt[:, g * HALF : (g + 1) * HALF], vps[:, :], lt[:, :], amax
            )

        # Attention.
        q_t, k_t, pe_t = acts["q"], acts["k"], acts["p"]
        out_ps = psO.tile([128, L], F32, tag="ops", name="out_ps")
        for blk in range(NB):
            pblk = ppool.tile([128, L], F32R, tag="pblk", name="pblk")
            zz = zpool.tile([128, 2], F32, tag="zz", name="zz")
            for h in range(2):
                sps = psA.tile([128, HALF], F32, tag="ps", name="sps")
                for n in range(2):
                    c0 = h * HALF + n * 512
                    nc.tensor.matmul(
                        sps[:, n * 512 : n * 512 + 512],
                        q_t[:, blk * 128 : blk * 128 + 128],
                        k_t[:, c0 : c0 + 512],
                        start=True,
                        stop=True,
                    )
                nc.scalar.activation(
                    pblk[:, h * HALF : (h + 1) * HALF],
                    sps[:, :],
                    Exp,
                    accum_out=zz[:, h : h + 1],
                )
            z = zpool.tile([128, 1], F32, tag="z", name="z")
            nc.vector.tensor_tensor(z[:, :], zz[:, 0:1], zz[:, 1:2], mybir.AluOpType.add)
            r = zpool.tile([128, 1], F32, tag="r", name="r")
            nc.vector.reciprocal(r[:, :], z[:, :])
            vts = vpool.tile([128, 128], F32R, tag="vts", name="vts")
            nc.vector.tensor_scalar_mul(
                vts[:, :], vt[:, blk * 128 : blk * 128 + 128], r[:, :]
            )
            for n in range(4):
                nc.tensor.matmul(
                    out_ps[:, n * 512 : n * 512 + 512],
                    vts[:, :],
                    pblk[:, n * 512 : n * 512 + 512],
                    start=(blk == 0),
                    stop=(blk == NB - 1),
                )

        outs = opool.tile([128, L], F32, tag="outs", name="outs")
        nc.vector.tensor_tensor(outs[:, :], out_ps[:, :], pe_t[:, :].bitcast(F32), mybir.AluOpType.add)
        nc.sync.dma_start(out=out[s, :, :], in_=outs[:, :])


def build():
    nc = bacc.Bacc("TRN2", target_bir_lowering=False, debug=False)
    x_d = nc.dram_tensor("x", [BP, CIN, L], F32, kind="ExternalInput")
    w1_d, w2_d = {}, {}
    for p in "qkvp":
        w1_d[p] = nc.dram_tensor(f"w1_{p}", [128, COUT], F32, kind="ExternalInput")
        w2_d[p] = nc.dram_tensor(f"w2_{p}", [CIN + 1, COUT], F32, kind="ExternalInput")
    zc_d = nc.dram_tensor("zc", [CIN, 1], F32, kind="ExternalInput")
    ones_d = nc.dram_tensor("onesrow", [1, L], F32, kind="ExternalInput")
    out_d = nc.dram_tensor("out", [BP, COUT, L], F32, kind="ExternalOutput")

    with tile.TileContext(nc) as tc, ExitStack() as ctx:
        _body(
            ctx,
            tc,
            x_d.ap(),
            {p: w1_d[p].ap() for p in "qkvp"},
            {p: w2_d[p].ap() for p in "qkvp"},
            zc_d.ap(),
            ones_d.ap(),
            out_d.ap(),
        )
    nc.compile()
    return nc


def _fold_weights(w, b, gamma, beta, mean, var):
    """Fold BN affine (fixed mean/var) into conv weights; return im2col chunks."""
    w = np.asarray(w, np.float64)
    scale = np.asarray(gamma, np.float64) / np.sqrt(np.asarray(var, np.float64) + EPS)
    shift = np.asarray(beta, np.float64) - np.asarray(mean, np.float64) * scale
    wf = w * scale[:, None, None]  # [COUT, CIN, K]
    bf = np.asarray(b, np.float64) * scale + shift
    w1 = np.empty((128, COUT), np.float32)
    w1[0:CIN] = wf[:, :, 0].T
    w1[CIN:128] = wf[:, :, 1].T
    w2 = np.empty((CIN + 1, COUT), np.float32)
    w2[0:CIN] = wf[:, :, 2].T
    w2[CIN] = bf
    return w1, w2


def _get_nc():
    if "nc" not in _CACHE:
        _CACHE["nc"] = build()
    return _CACHE["nc"]


def make_in_maps(inputs):
    x = np.ascontiguousarray(np.asarray(inputs["x"], np.float32))
    folded = {}
    for p in "qkvp":
        key = p if p != "p" else "pe"
        folded[p] = _fold_weights(
            inputs[f"{key}_w"],
            inputs[f"{key}_b"],
            inputs[f"{key}_gamma"],
            inputs[f"{key}_beta"],
            inputs[f"{key}_mean"],
            inputs[f"{key}_var"],
        )
    in_maps = []
    for i in range(NCORES):
        m = {"x": np.ascontiguousarray(x[i * BP : (i + 1) * BP])}
        for p in "qkvp":
            m[f"w1_{p}"] = folded[p][0]
            m[f"w2_{p}"] = folded[p][1]
        m["zc"] = np.zeros((CIN, 1), np.float32)
        m["onesrow"] = np.ones((1, L), np.float32)
        in_maps.append(m)
    return in_maps


def kernel(**inputs):
    nc = _get_nc()
    in_maps = make_in_maps(inputs)
    res = run_bass_kernel_spmd(nc, in_maps, core_ids=list(range(NCORES)))
    out = np.concatenate([res.results[i]["out"] for i in range(NCORES)], axis=0)
    return out.astype(np.float32)


if __name__ == "__main__":
    rng = np.random.default_rng(0)
    ins = {"x": rng.standard_normal((B, CIN, L), dtype=np.float32)}
    for p in ("q", "k", "v", "pe"):
        ins[f"{p}_w"] = (rng.standard_normal((COUT, CIN, KW)) * 0.05).astype(np.float32)
        ins[f"{p}_b"] = (rng.standard_normal(COUT) * 0.05).astype(np.float32)
        ins[f"{p}_gamma"] = rng.uniform(0.5, 1.5, COUT).astype(np.float32)
        ins[f"{p}_beta"] = (rng.standard_normal(COUT) * 0.05).astype(np.float32)
        ins[f"{p}_mean"] = (rng.standard_normal(COUT) * 0.05).astype(np.float32)
        ins[f"{p}_var"] = rng.uniform(0.5, 1.5, COUT).astype(np.float32)
    got = kernel(**ins)
    print("kernel output:", got.shape, got.dtype, np.abs(got).mean())


# revision 9
# speedup vs baseline: 1.0561x; 1.0561x over previous
"""Trainium2 Bass kernel for nn_Conv1dAttention.

Math (per sample):
  q,k,v,pe = lrelu(bn(conv1d(x, W_p)))           # [C=128, L=2048], Cin=64, K=3
  S = q^T k                                      # [L, L]
  P = softmax_rows(S)                            # softmax over last axis
  out = v @ P + pe                               # [C, L]

Sharding: data-parallel over batch B=16 across 8 NeuronCores (2 samples/core).
Same NEFF on all cores, per-core input shards, no collectives.

Design notes:
  - BN (uses given mean/var, not data stats) is folded into conv weights and
    bias on the host. Bias is injected via an appended ones-row in the im2col
    tile, so conv+bias is pure matmul.
  - im2col: contraction 192 = two chunks: chunk1 = 128 rows (k=0 shifted,
    k=1 center), chunk2 = 65 rows (k=2 shifted + ones row for bias).
  - Q, K, PE computed in [c, l] layout (N=512 matmuls). V computed directly
    transposed [l, c] (stationary = im2col slice) to feed the V@P matmul
    without PE transposes.
  - Softmax without max subtraction (logits bounded ~60 for this model's
    weight scale; exp stays finite in fp32). exp on ScalarE with accum_out
    giving row sums; 1/Z folded into the tiny [128,128] V^T block on DVE.
  - All matmuls in float32r (FP22, 1 cycle/row) — ample precision here.
  - PSUM: 4 banks = [128,2048] output accumulator; 4 banks = two rotating
    [128,1024] tiles shared by convs and S-chunks.
"""

import sys

if "/opt/trn_rl_repo" not in sys.path:
    sys.path.insert(0, "/opt/trn_rl_repo")

from contextlib import ExitStack

import numpy as np

import concourse.bass as bass
import concourse.tile as tile
from concourse import bacc, mybir
from concourse.bass_utils import run_bass_kernel_spmd

B, CIN, COUT, KW, L = 16, 64, 128, 3, 2048
NCORES = 8
BP = B // NCORES  # samples per core
EPS = 1e-5
SLOPE = 0.3
F32 = mybir.dt.float32
F32R = mybir.dt.float32r
NB = L // 128  # 16 a-blocks
HALF = 1024

_CACHE = {}


def _r(ap):
    return ap.bitcast(F32R)


def _body(ctx, tc, x, w1, w2, zc, onesrow, out):
    nc = tc.nc
    amax = mybir.AluOpType.max
    Exp = mybir.ActivationFunctionType.Exp
    Lrelu = mybir.ActivationFunctionType.Lrelu

    wpool = ctx.enter_context(tc.tile_pool(name="wpool", bufs=1))
    xpool = ctx.enter_context(tc.tile_pool(name="xpool", bufs=2))
    apool = ctx.enter_context(tc.tile_pool(name="apool", bufs=2))
    ppool = ctx.enter_context(tc.tile_pool(name="ppool", bufs=2))
    opool = ctx.enter_context(tc.tile_pool(name="opool", bufs=2))
    vpool = ctx.enter_context(tc.tile_pool(name="vpool", bufs=2))
    zpool = ctx.enter_context(tc.tile_pool(name="zpool", bufs=4))
    lpool = ctx.enter_context(tc.tile_pool(name="lpool", bufs=2))
    psA = ctx.enter_context(tc.tile_pool(name="psA", bufs=2, space="PSUM"))
    psO = ctx.enter_context(tc.tile_pool(name="psO", bufs=1, space="PSUM"))

    w1_t, w2_t = {}, {}
    for p in "qkvp":
        w1_t[p] = wpool.tile([128, COUT], F32R, tag=f"w1{p}", name=f"w1{p}")
        nc.sync.dma_start(out=w1_t[p][:, :], in_=w1[p][:, :].bitcast(F32R))
        w2_t[p] = wpool.tile([CIN + 1, COUT], F32R, tag=f"w2{p}", name=f"w2{p}")
        nc.sync.dma_start(out=w2_t[p][:, :], in_=w2[p][:, :].bitcast(F32R))

    def emit_xs(s):
        # im2col tiles.
        # xs1 rows 0-63  = x[cin, l-1]  (k=0), rows 64-127 = x[cin, l] (k=1)
        # xs2 rows 0-63  = x[cin, l+1]  (k=2), row 64 = ones (bias)
        xs1 = xpool.tile([128, L], F32R, tag="xs1", name="xs1")
        nc.sync.dma_start(out=xs1[0:CIN, 1:L], in_=x[s, :, 0 : L - 1].bitcast(F32R))
        nc.sync.dma_start(out=xs1[0:CIN, 0:1], in_=zc[:, :].bitcast(F32R))
        nc.sync.dma_start(out=xs1[CIN:128, 0:L], in_=x[s, :, :].bitcast(F32R))
        xs2 = xpool.tile([CIN + 1, L], F32R, tag="xs2", name="xs2")
        nc.sync.dma_start(out=xs2[0:CIN, 0 : L - 1], in_=x[s, :, 1:L].bitcast(F32R))
        nc.sync.dma_start(out=xs2[0:CIN, L - 1 : L], in_=zc[:, :].bitcast(F32R))
        nc.sync.dma_start(out=xs2[CIN : CIN + 1, :], in_=onesrow[:, :].bitcast(F32R))
        return xs1, xs2

    def lrelu_drain(dst_ap, ps_ap, on_act):
        # lrelu(y) = y + (1-slope)*relu(-y)
        if on_act:
            # ACT Lrelu's table has a hardwired 0.01 slope (alpha ignored),
            # so split: relu(-y) on ACT, then one fused DVE op.
            lt = lpool.tile([128, HALF], F32, tag="lt", name="lt")
            nc.scalar.activation(
                lt[:, :], ps_ap, mybir.ActivationFunctionType.Relu, scale=-1.0
            )
            nc.vector.scalar_tensor_tensor(
                dst_ap,
                lt[:, :],
                1.0 - SLOPE,
                ps_ap,
                op0=mybir.AluOpType.mult,
                op1=mybir.AluOpType.add,
            )
        else:
            lt = lpool.tile([128, HALF], F32, tag="lt", name="lt")
            nc.vector.tensor_scalar_mul(lt[:, :], ps_ap, SLOPE)
            nc.vector.tensor_tensor(dst_ap, ps_ap, lt[:, :], amax)

    def conv_chunk(xs1, xs2, p, dst, h, on_act):
        # one [128,1024] half of a [c, l]-layout conv
        cps = psA.tile([128, HALF], F32, tag="ps", name="cps")
        for n in range(2):
            c0 = h * HALF + n * 512
            nc.tensor.matmul(
                cps[:, n * 512 : n * 512 + 512],
                w1_t[p][:, :],
                xs1[:, c0 : c0 + 512],
                start=True,
                stop=False,
            )
            nc.tensor.matmul(
                cps[:, n * 512 : n * 512 + 512],
                w2_t[p][:, :],
                xs2[:, c0 : c0 + 512],
                start=False,
                stop=True,
            )
        lrelu_drain(dst[:, h * HALF : (h + 1) * HALF], cps[:, :], on_act)

    def vt_group(xs1, xs2, vt, g, on_act):
        # 8 l-blocks of V in transposed [l, c] layout -> one [128,1024] tile
        vps = psA.tile([128, HALF], F32, tag="ps", name="vps")
        for i in range(8):
            blk = g * 8 + i
            lsl = slice(blk * 128, blk * 128 + 128)
            pc = slice(i * 128, i * 128 + 128)
            nc.tensor.matmul(
                vps[:, pc], xs1[:, lsl], w1_t["v"][:, :], start=True, stop=False
            )
            nc.tensor.matmul(
                vps[:, pc], xs2[:, lsl], w2_t["v"][:, :], start=False, stop=True
            )
        lrelu_drain(vt[:, g * HALF : (g + 1) * HALF], vps[:, :], on_act)

    def make_conv_units(s, xs1, xs2, on_act):
        """Return (tiles, unit-thunks). Order: Q, K, VT halves first (needed
        from attention block 0), PE conv last (needed only at the end)."""
        q_t = apool.tile([128, L], F32R, tag="actq", name="actq")
        k_t = apool.tile([128, L], F32R, tag="actk", name="actk")
        pe_t = apool.tile([128, L], F32R, tag="actp", name="actp")
        vt = apool.tile([128, L], F32, tag="vt", name="vt")
        units = []
        for h in range(2):
            units.append(lambda h=h: conv_chunk(xs1, xs2, "q", q_t, h, on_act))
        for h in range(2):
            units.append(lambda h=h: conv_chunk(xs1, xs2, "k", k_t, h, on_act))
        for g in range(2):
            units.append(lambda g=g: vt_group(xs1, xs2, vt, g, on_act))
        for h in range(2):
            units.append(lambda h=h: conv_chunk(xs1, xs2, "p", pe_t, h, on_act))
        return (q_t, k_t, pe_t, vt), units

    def attn_block(tiles, out_ps, blk):
        q_t, k_t, pe_t, vt = tiles
        pblk = ppool.tile([128, L], F32R, tag="pblk", name="pblk")
        zz = zpool.tile([128, 2], F32, tag="zz", name="zz")
        for h in range(2):
            sps = psA.tile([128, HALF], F32, tag="ps", name="sps")
            for n in range(2):
                c0 = h * HALF + n * 512
                nc.tensor.matmul(
                    sps[:, n * 512 : n * 512 + 512],
                    q_t[:, blk * 128 : blk * 128 + 128],
                    k_t[:, c0 : c0 + 512],
                    start=True,
                    stop=True,
                )
            nc.scalar.activation(
                pblk[:, h * HALF : (h + 1) * HALF],
                sps[:, :],
                Exp,
                accum_out=zz[:, h : h + 1],
            )
        z = zpool.tile([128, 1], F32, tag="z", name="z")
        nc.vector.tensor_tensor(z[:, :], zz[:, 0:1], zz[:, 1:2], mybir.AluOpType.add)
        r = zpool.tile([128, 1], F32, tag="r", name="r")
        nc.vector.reciprocal(r[:, :], z[:, :])
        vts = vpool.tile([128, 128], F32R, tag="vts", name="vts")
        nc.vector.tensor_scalar_mul(
            vts[:, :], vt[:, blk * 128 : blk * 128 + 128], r[:, :]
        )
        for n in range(4):
            nc.tensor.matmul(
                out_ps[:, n * 512 : n * 512 + 512],
                vts[:, :],
                pblk[:, n * 512 : n * 512 + 512],
                start=(blk == 0),
                stop=(blk == NB - 1),
            )

    def finish_sample(tiles, out_ps, s):
        pe_t = tiles[2]
        outs = opool.tile([128, L], F32, tag="outs", name="outs")
        nc.vector.tensor_tensor(
            outs[:, :], out_ps[:, :], pe_t[:, :].bitcast(F32), mybir.AluOpType.add
        )
        nc.sync.dma_start(out=out[s, :, :], in_=outs[:, :])

    assert BP == 2
    # Phase A: sample 0 convs, LReLU on ScalarE (idle during convs; keeps
    # the DVE free and the psum drain fast so PE stays dense).
    xs0 = emit_xs(0)
    tiles0, units0 = make_conv_units(0, *xs0, on_act=True)
    for u in units0:
        u()
    # Phase B: sample 0 attention (ScalarE-bound) with sample 1 convs
    # interleaved (PE + DVE filler).
    xs1_ = emit_xs(1)
    tiles1, units1 = make_conv_units(1, *xs1_, on_act=False)
    out_ps0 = psO.tile([128, L], F32, tag="ops", name="out_ps0")
    ui = 0
    for blk in range(NB):
        attn_block(tiles0, out_ps0, blk)
        while ui < len(units1) and ui * NB <= blk * len(units1):
            units1[ui]()
            ui += 1
    while ui < len(units1):
        units1[ui]()
        ui += 1
    finish_sample(tiles0, out_ps0, 0)
    # Phase C: sample 1 attention.
    out_ps1 = psO.tile([128, L], F32, tag="ops", name="out_ps1")
    for blk in range(NB):
        attn_block(tiles1, out_ps1, blk)
    finish_sample(tiles1, out_ps1, 1)


def build():
    nc = bacc.Bacc("TRN2", target_bir_lowering=False, debug=False)
    x_d = nc.dram_tensor("x", [BP, CIN, L], F32, kind="ExternalInput")
    w1_d, w2_d = {}, {}
    for p in "qkvp":
        w1_d[p] = nc.dram_tensor(f"w1_{p}", [128, COUT], F32, kind="ExternalInput")
        w2_d[p] = nc.dram_tensor(f"w2_{p}", [CIN + 1, COUT], F32, kind="ExternalInput")
    zc_d = nc.dram_tensor("zc", [CIN, 1], F32, kind="ExternalInput")
    ones_d = nc.dram_tensor("onesrow", [1, L], F32, kind="ExternalInput")
    out_d = nc.dram_tensor("out", [BP, COUT, L], F32, kind="ExternalOutput")

    with tile.TileContext(nc) as tc, ExitStack() as ctx:
        _body(
            ctx,
            tc,
            x_d.ap(),
            {p: w1_d[p].ap() for p in "qkvp"},
            {p: w2_d[p].ap() for p in "qkvp"},
            zc_d.ap(),
            ones_d.ap(),
            out_d.ap(),
        )
    nc.compile()
    return nc


def _fold_weights(w, b, gamma, beta, mean, var):
    """Fold BN affine (fixed mean/var) into conv weights; return im2col chunks."""
    w = np.asarray(w, np.float64)
    scale = np.asarray(gamma, np.float64) / np.sqrt(np.asarray(var, np.float64) + EPS)
    shift = np.asarray(beta, np.float64) - np.asarray(mean, np.float64) * scale
    wf = w * scale[:, None, None]  # [COUT, CIN, K]
    bf = np.asarray(b, np.float64) * scale + shift
    w1 = np.empty((128, COUT), np.float32)
    w1[0:CIN] = wf[:, :, 0].T
    w1[CIN:128] = wf[:, :, 1].T
    w2 = np.empty((CIN + 1, COUT), np.float32)
    w2[0:CIN] = wf[:, :, 2].T
    w2[CIN] = bf
    return w1, w2


def _get_nc():
    if "nc" not in _CACHE:
        _CACHE["nc"] = build()
    return _CACHE["nc"]


def make_in_maps(inputs):
    x = np.ascontiguousarray(np.asarray(inputs["x"], np.float32))
    folded = {}
    for p in "qkvp":
        key = p if p != "p" else "pe"
        folded[p] = _fold_weights(
            inputs[f"{key}_w"],
            inputs[f"{key}_b"],
            inputs[f"{key}_gamma"],
            inputs[f"{key}_beta"],
            inputs[f"{key}_mean"],
            inputs[f"{key}_var"],
        )
    in_maps = []
    for i in range(NCORES):
        m = {"x": np.ascontiguousarray(x[i * BP : (i + 1) * BP])}
        for p in "qkvp":
            m[f"w1_{p}"] = folded[p][0]
            m[f"w2_{p}"] = folded[p][1]
        m["zc"] = np.zeros((CIN, 1), np.float32)
        m["onesrow"] = np.ones((1, L), np.float32)
        in_maps.append(m)
    return in_maps


def kernel(**inputs):
    nc = _get_nc()
    in_maps = make_in_maps(inputs)
    res = run_bass_kernel_spmd(nc, in_maps, core_ids=list(range(NCORES)))
    out = np.concatenate([res.results[i]["out"] for i in range(NCORES)], axis=0)
    return out.astype(np.float32)


if __name__ == "__main__":
    rng = np.random.default_rng(0)
    ins = {"x": rng.standard_normal((B, CIN, L), dtype=np.float32)}
    for p in ("q", "k", "v", "pe"):
        ins[f"{p}_w"] = (rng.standard_normal((COUT, CIN, KW)) * 0.05).astype(np.float32)
        ins[f"{p}_b"] = (rng.standard_normal(COUT) * 0.05).astype(np.float32)
        ins[f"{p}_gamma"] = rng.uniform(0.5, 1.5, COUT).astype(np.float32)
        ins[f"{p}_beta"] = (rng.standard_normal(COUT) * 0.05).astype(np.float32)
        ins[f"{p}_mean"] = (rng.standard_normal(COUT) * 0.05).astype(np.float32)
        ins[f"{p}_var"] = rng.uniform(0.5, 1.5, COUT).astype(np.float32)
    got = kernel(**ins)
    print("kernel output:", got.shape, got.dtype, np.abs(got).mean())
